# revision 21
# baseline (speedup 1.0000x reference)
"""Causal multi-head self-attention with RoPE on 8 Trainium2 NeuronCores.

Problem: x[2,2048,2048] fp32, wq/wk/wv[2048,2048] fp32 (Linear [out,in]),
H=16 heads, dk=128, causal softmax attention, RoPE(theta=1e4).

Sharding (hybrid tensor/data parallel, no collectives): core c handles
batch b=c//4 and head group hg=c%4 (4 heads = 512 output features).
Each core gets a transposed bf16 copy of x[b] plus its weight column slice;
the host concatenates the 8 per-core outputs.

Device kernel (per core, all matmuls bf16 with fp32 PSUM accumulation —
fp32 matmul is 1/4 rate on TRN2):
  - QKV projections: qT/kT in [dk, t] layout (lhsT = w^T k-tile, rhs = x^T
    t-block), v in [t, dk] layout with a ones-column appended (129 wide) so
    the attention output matmul also produces the softmax denominator.
  - RoPE via rotate-half: Qrot = Q*C2 + (perm^T @ Q)*S2 where perm is a
    constant +-1 pair-swap matrix applied on the tensor engine (vector
    engines cannot cross partitions). 1/sqrt(dk) is folded into wq on host.
  - Attention in s^T = [keys, queries] layout: per (head, 512-query block),
    key tiles of 128; scores matmul pairs write a 2-bank PSUM tile so one
    ScalarE exp call covers 1024 columns. Causality: key tiles above the
    diagonal are skipped entirely; diagonal tiles multiply a constant
    [128,128] triangle mask into p^T and skip fully-masked query subtiles.
  - out[q,dk] accumulates pv matmuls (lhsT = p^T q-subtile, rhs = v_ext),
    then one reciprocal + per-partition scale normalizes, DMA to DRAM fp32.
"""
import os
import sys
import time

# a wedged device from a prior process recovers on reset; must be set
# before the first jax/neuron import in this process
os.environ.setdefault("NEURON_RT_RESET_CORES", "1")

sys.path.insert(0, "/opt/trn_rl_repo")

import numpy as np
import ml_dtypes

import concourse.bass as bass
import concourse.bacc as bacc
import concourse.mybir as mybir
import concourse.tile as tile
from concourse.bass_utils import run_bass_kernel_spmd

B, S, D = 2, 2048, 2048
H, DK = 16, 128
N_CORES = 8
HPC = 4            # heads per core
FPC = HPC * DK     # features per core (512)
P = 128            # partitions
KT = D // P        # contraction k-tiles (16)
TBW = 512          # token-block width for projections
NTB = S // TBW     # 4 t-blocks
NQT = S // P       # 16 query tiles of 128
THETA = 10000.0

bf16 = ml_dtypes.bfloat16
_mult = mybir.AluOpType.mult
_add = mybir.AluOpType.add
_subtract = mybir.AluOpType.subtract

_PROGRAM_CACHE = {}


def _build_program():
    dt = mybir.dt
    nc = bacc.Bacc("TRN2", target_bir_lowering=False, debug=False,
                   num_devices=N_CORES)

    xT_d = nc.dram_tensor("xT", [D, S], dt.bfloat16, kind="ExternalInput").ap()
    wqT_d = nc.dram_tensor("wqT", [D, FPC], dt.bfloat16, kind="ExternalInput").ap()
    wkT_d = nc.dram_tensor("wkT", [D, FPC], dt.bfloat16, kind="ExternalInput").ap()
    wvT_d = nc.dram_tensor("wvT", [D, FPC], dt.bfloat16, kind="ExternalInput").ap()
    c2_d = nc.dram_tensor("c2", [P, S], dt.bfloat16, kind="ExternalInput").ap()
    s2_d = nc.dram_tensor("s2", [P, S], dt.bfloat16, kind="ExternalInput").ap()
    perm_d = nc.dram_tensor("perm", [P, P], dt.bfloat16, kind="ExternalInput").ap()
    tri_d = nc.dram_tensor("tri", [P, P], dt.bfloat16, kind="ExternalInput").ap()
    out_d = nc.dram_tensor("out", [S, FPC], dt.float32, kind="ExternalOutput").ap()

    ts = bass.ts

    with tile.TileContext(nc) as tc:
        with (
            tc.tile_pool(name="const", bufs=1) as cpool,
            tc.tile_pool(name="work", bufs=4) as wpool,
            tc.tile_pool(name="small", bufs=6) as smpool,
            tc.tile_pool(name="ppsum", bufs=2,
                         space=bass.MemorySpace.PSUM) as ppsum,
            tc.tile_pool(name="spsum", bufs=3,
                         space=bass.MemorySpace.PSUM) as spsum,
            tc.tile_pool(name="opsum", bufs=1,
                         space=bass.MemorySpace.PSUM) as opsum,
            tc.tile_pool(name="ppool", bufs=8) as ppool,
        ):
            # --- persistent SBUF tensors ---
            wq_sb = cpool.tile([P, KT, FPC], dt.bfloat16, tag="wq")
            wk_sb = cpool.tile([P, KT, FPC], dt.bfloat16, tag="wk")
            wv_sb = cpool.tile([P, KT, FPC], dt.bfloat16, tag="wv")
            c2_sb = cpool.tile([P, S], dt.bfloat16, tag="c2")
            s2_sb = cpool.tile([P, S], dt.bfloat16, tag="s2")
            perm_sb = cpool.tile([P, P], dt.bfloat16, tag="perm")
            tri_sb = cpool.tile([P, P], dt.bfloat16, tag="tri")
            qT_sb = cpool.tile([P, HPC * S], dt.bfloat16, tag="qT")
            kT_sb = cpool.tile([P, HPC * S], dt.bfloat16, tag="kT")
            # v with ones column: index n = h*NQT + tt -> [128 tokens, 129]
            v_sb = cpool.tile([P, HPC * NQT * (DK + 1)], dt.bfloat16, tag="v")
            v_rr = v_sb.rearrange("p (n c) -> p n c", c=DK + 1)
            v_hr = v_sb.rearrange("p (h t c) -> p t h c", h=HPC, c=DK + 1)
            xts = [cpool.tile([P, KT, TBW], dt.bfloat16,
                              tag=f"xt{i}", name=f"xt{i}") for i in range(NTB)]

            # initial loads spread across engine queues for parallelism;
            # wq + xt0 first so the first projection chain starts early
            xT_r = xT_d.rearrange("(kt p) t -> p kt t", p=P)
            # wave 1: only what the first projection chain needs (wq head-0
            # columns + x t-block 0) so it starts ~12us in; everything else
            # queues behind it on the three DMA-capable engines
            wq_r = wqT_d.rearrange("(kt p) o -> p kt o", p=P)
            wk_r = wkT_d.rearrange("(kt p) o -> p kt o", p=P)
            wv_r = wvT_d.rearrange("(kt p) o -> p kt o", p=P)
            # x tiles split column-wise across the two HWDGE queues so each
            # tile completes in half the time; later-needed weights queue last
            HW = TBW // 2
            nc.sync.dma_start(wq_sb[:, :, 0:DK], wq_r[:, :, 0:DK])
            nc.scalar.dma_start(xts[0][:, :, 0:HW], xT_r[:, :, 0:HW])
            nc.sync.dma_start(xts[0][:, :, HW:TBW], xT_r[:, :, HW:TBW])
            nc.gpsimd.dma_start(c2_sb[:], c2_d[:])
            nc.gpsimd.dma_start(s2_sb[:], s2_d[:])
            nc.gpsimd.dma_start(perm_sb[:], perm_d[:])
            nc.gpsimd.dma_start(tri_sb[:], tri_d[:])
            nc.sync.dma_start(wk_sb[:, :, 0:DK], wk_r[:, :, 0:DK])
            for i in range(1, NTB):
                lo = i * TBW
                nc.scalar.dma_start(xts[i][:, :, 0:HW], xT_r[:, :, lo:lo + HW])
                nc.sync.dma_start(xts[i][:, :, HW:TBW],
                                  xT_r[:, :, lo + HW:lo + TBW])
            nc.scalar.dma_start(wk_sb[:, :, DK:FPC], wk_r[:, :, DK:FPC])
            nc.sync.dma_start(wv_sb[:], wv_r[:])
            nc.scalar.dma_start(wq_sb[:, :, DK:FPC], wq_r[:, :, DK:FPC])
            nc.gpsimd.memset(v_rr[:, :, DK:DK + 1], 1.0)

            # PE warm-up: dependency-free matmuls spin the tensor engine out
            # of its cold HAM state while the first input DMAs land
            warm_sb = cpool.tile([P, P], dt.bfloat16, tag="warm")
            nc.gpsimd.memset(warm_sb[:], 0.0)
            warm_ps = spsum.tile([P, P], dt.float32, tag="qtp",
                                 name="warm_ps", bufs=1)
            for _ in range(40):
                nc.tensor.matmul(warm_ps[:], warm_sb[:], warm_sb[:],
                                 start=True, stop=True)

            def v_proj():
                for tb in range(NTB):
                    for tsub in range(4):
                        tt = tb * 4 + tsub
                        vps = ppsum.tile([P, FPC], dt.float32,
                                         tag="qps", name="vps")
                        for kt in range(KT):
                            nc.tensor.matmul(
                                vps[:],
                                xts[tb][:, kt, ts(tsub, P)],
                                wv_sb[:, kt, :],
                                start=(kt == 0), stop=(kt == KT - 1),
                            )
                        nc.scalar.copy(
                            v_hr[:, tt, :, 0:DK],
                            vps.rearrange("p (h c) -> p h c", h=HPC))

            def qk_proj(h):
                hq = h * S
                for tb in range(NTB):
                    for wsb, dest in ((wq_sb, qT_sb), (wk_sb, kT_sb)):
                        qps = ppsum.tile([P, TBW], dt.float32,
                                         tag="qps", name="qps")
                        for kt in range(KT):
                            nc.tensor.matmul(
                                qps[:],
                                wsb[:, kt, ts(h, DK)],
                                xts[tb][:, kt, :],
                                start=(kt == 0), stop=(kt == KT - 1),
                            )
                        qb_ = wpool.tile([P, TBW], dt.bfloat16,
                                         tag="qb", name="qb")
                        nc.scalar.copy(qb_[:], qps[:])
                        qtp = spsum.tile([P, TBW], dt.float32,
                                         tag="qtp", name="qtp", bufs=1)
                        nc.tensor.matmul(qtp[:], perm_sb[:], qb_[:],
                                         start=True, stop=True)
                        t1 = wpool.tile([P, TBW], dt.bfloat16,
                                        tag="t1", name="t1")
                        nc.vector.scalar_tensor_tensor(
                            t1[:], qb_[:], 1.0, c2_sb[:, ts(tb, TBW)],
                            op0=_mult, op1=_mult)
                        t2 = wpool.tile([P, TBW], dt.bfloat16,
                                        tag="t2", name="t2")
                        nc.vector.scalar_tensor_tensor(
                            t2[:], qtp[:], 1.0, s2_sb[:, ts(tb, TBW)],
                            op0=_mult, op1=_mult)
                        nc.vector.scalar_tensor_tensor(
                            dest[:, hq + tb * TBW:hq + (tb + 1) * TBW],
                            t1[:], 1.0, t2[:], op0=_mult, op1=_add)

            def attention(h):
                hq = h * S
                for qb in range(NTB):
                    # two accumulators share a PSUM bank: one start pending-
                    # zeroes the whole 2KB zero region, one stop (on the
                    # higher subtile, which always finishes later) releases it
                    o01 = opsum.tile([P, 2, DK + 1], dt.float32,
                                     tag="o01", name="o01")
                    o23 = opsum.tile([P, 2, DK + 1], dt.float32,
                                     tag="o23", name="o23")
                    oap = [o01[:, 0, :], o01[:, 1, :],
                           o23[:, 0, :], o23[:, 1, :]]
                    for kt in range(4 * qb + 4):
                        od = kt - 4 * qb  # >=0 on diagonal tiles
                        lo = max(od, 0) * P
                        sps = spsum.tile([P, TBW], dt.float32,
                                         tag="sps", name="sps")
                        nc.tensor.matmul(
                            sps[:, lo:TBW],
                            kT_sb[:, hq + kt * P:hq + (kt + 1) * P],
                            qT_sb[:, hq + qb * TBW + lo:hq + (qb + 1) * TBW],
                            start=True, stop=True)
                        pT = ppool.tile([P, TBW], dt.bfloat16,
                                        tag="pT", name="pT")
                        nc.scalar.activation(
                            pT[:, lo:TBW], sps[:, lo:TBW],
                            mybir.ActivationFunctionType.Exp)
                        if od >= 0:
                            nc.vector.scalar_tensor_tensor(
                                pT[:, od * P:(od + 1) * P],
                                pT[:, od * P:(od + 1) * P],
                                1.0, tri_sb[:], op0=_mult, op1=_mult)
                        for osub in range(max(od, 0), 4):
                            qt = 4 * qb + osub
                            nc.tensor.matmul(
                                oap[osub],
                                pT[:, osub * P:(osub + 1) * P],
                                v_rr[:, h * NQT + kt, :],
                                start=(kt == 0 and osub % 2 == 0),
                                stop=(kt == qt and osub % 2 == 1))
                    recs = []
                    for pair, ot in ((0, o01), (1, o23)):
                        rec = smpool.tile([P, 2], dt.float32,
                                          tag="rec", name="rec")
                        nc.vector.reciprocal(rec[:], ot[:, :, DK])
                        recs.append(rec)
                    for osub in range(4):
                        qt = 4 * qb + osub
                        osb = smpool.tile([P, DK], dt.float32,
                                          tag="osb", name="osb")
                        nc.vector.tensor_scalar_mul(
                            osb[:], oap[osub][:, 0:DK],
                            recs[osub // 2][:, osub % 2:osub % 2 + 1])
                        nc.sync.dma_start(out_d[ts(qt, P), ts(h, DK)], osb[:])

            qk_proj(0)
            v_proj()
            attention(0)
            for h in range(1, HPC):
                qk_proj(h)
                attention(h)

    nc.compile()
    return nc


def _host_tables():
    pos = np.arange(S, dtype=np.float64)
    i = np.arange(DK // 2, dtype=np.float64)
    inv_freq = THETA ** (-2.0 * i / DK)
    ang = pos[None, :] * inv_freq[:, None]          # [64, S]
    c2 = np.repeat(np.cos(ang), 2, axis=0).astype(bf16)   # [128, S]
    s2 = np.repeat(np.sin(ang), 2, axis=0).astype(bf16)
    perm = np.zeros((P, P), np.float32)
    idx = np.arange(DK // 2)
    perm[2 * idx + 1, 2 * idx] = -1.0
    perm[2 * idx, 2 * idx + 1] = 1.0
    tri = (np.arange(P)[:, None] <= np.arange(P)[None, :]).astype(np.float32)
    return c2, s2, perm.astype(bf16), tri.astype(bf16)


def kernel(x, wq, wk, wv):
    x = np.asarray(x, dtype=np.float32)
    wq = np.asarray(wq, dtype=np.float32)
    wk = np.asarray(wk, dtype=np.float32)
    wv = np.asarray(wv, dtype=np.float32)

    if "nc" not in _PROGRAM_CACHE:
        _PROGRAM_CACHE["nc"] = _build_program()
    nc = _PROGRAM_CACHE["nc"]

    c2, s2, perm, tri = _host_tables()
    scale = np.float32(1.0 / np.sqrt(DK))

    in_maps = []
    for c in range(N_CORES):
        b, hg = divmod(c, HPC)
        rows = slice(hg * FPC, (hg + 1) * FPC)
        in_maps.append({
            "xT": np.ascontiguousarray(x[b].T).astype(bf16),
            "wqT": np.ascontiguousarray((wq[rows] * scale).T).astype(bf16),
            "wkT": np.ascontiguousarray(wk[rows].T).astype(bf16),
            "wvT": np.ascontiguousarray(wv[rows].T).astype(bf16),
            "c2": c2, "s2": s2, "perm": perm, "tri": tri,
        })

    last_err = None
    for attempt in range(3):
        try:
            res = run_bass_kernel_spmd(nc, in_maps, list(range(N_CORES)),
                                       **_PROGRAM_CACHE.get("run_kwargs", {}))
            break
        except Exception as e:  # transient NRT device errors recover on retry
            last_err = e
            time.sleep(2.0)
    else:
        raise last_err
    _PROGRAM_CACHE["last_results"] = res

    out = np.empty((B, S, D), np.float32)
    for c in range(N_CORES):
        b, hg = divmod(c, HPC)
        out[b, :, hg * FPC:(hg + 1) * FPC] = res.results[c]["out"]
    return out


# revision 23
# speedup vs baseline: 1.0040x; 1.0040x over previous
"""Causal multi-head self-attention with RoPE on 8 Trainium2 NeuronCores.

Problem: x[2,2048,2048] fp32, wq/wk/wv[2048,2048] fp32 (Linear [out,in]),
H=16 heads, dk=128, causal softmax attention, RoPE(theta=1e4).

Sharding (hybrid tensor/data parallel, no collectives): core c handles
batch b=c//4 and head group hg=c%4 (4 heads = 512 output features).
Each core gets a transposed bf16 copy of x[b] plus its weight column slice;
the host concatenates the 8 per-core outputs.

Device kernel (per core, all matmuls bf16 with fp32 PSUM accumulation —
fp32 matmul is 1/4 rate on TRN2):
  - QKV projections: qT/kT in [dk, t] layout (lhsT = w^T k-tile, rhs = x^T
    t-block), v in [t, dk] layout with a ones-column appended (129 wide) so
    the attention output matmul also produces the softmax denominator.
  - RoPE via rotate-half: Qrot = Q*C2 + (perm^T @ Q)*S2 where perm is a
    constant +-1 pair-swap matrix applied on the tensor engine (vector
    engines cannot cross partitions). 1/sqrt(dk) is folded into wq on host.
  - Attention in s^T = [keys, queries] layout: per (head, 512-query block),
    key tiles of 128; scores matmul pairs write a 2-bank PSUM tile so one
    ScalarE exp call covers 1024 columns. Causality: key tiles above the
    diagonal are skipped entirely; diagonal tiles multiply a constant
    [128,128] triangle mask into p^T and skip fully-masked query subtiles.
  - out[q,dk] accumulates pv matmuls (lhsT = p^T q-subtile, rhs = v_ext),
    then one reciprocal + per-partition scale normalizes, DMA to DRAM fp32.
"""
import os
import sys
import time

# a wedged device from a prior process recovers on reset; must be set
# before the first jax/neuron import in this process
os.environ.setdefault("NEURON_RT_RESET_CORES", "1")

sys.path.insert(0, "/opt/trn_rl_repo")

import numpy as np
import ml_dtypes

import concourse.bass as bass
import concourse.bacc as bacc
import concourse.mybir as mybir
import concourse.tile as tile
from concourse.bass_utils import run_bass_kernel_spmd

B, S, D = 2, 2048, 2048
H, DK = 16, 128
N_CORES = 8
HPC = 4            # heads per core
FPC = HPC * DK     # features per core (512)
P = 128            # partitions
KT = D // P        # contraction k-tiles (16)
TBW = 512          # token-block width for projections
NTB = S // TBW     # 4 t-blocks
NQT = S // P       # 16 query tiles of 128
THETA = 10000.0

bf16 = ml_dtypes.bfloat16
_mult = mybir.AluOpType.mult
_add = mybir.AluOpType.add
_subtract = mybir.AluOpType.subtract

_PROGRAM_CACHE = {}


def _build_program():
    dt = mybir.dt
    nc = bacc.Bacc("TRN2", target_bir_lowering=False, debug=False,
                   num_devices=N_CORES)

    xT_d = nc.dram_tensor("xT", [D, S], dt.bfloat16, kind="ExternalInput").ap()
    wqT_d = nc.dram_tensor("wqT", [D, FPC], dt.bfloat16, kind="ExternalInput").ap()
    wkT_d = nc.dram_tensor("wkT", [D, FPC], dt.bfloat16, kind="ExternalInput").ap()
    wvT_d = nc.dram_tensor("wvT", [D, FPC], dt.bfloat16, kind="ExternalInput").ap()
    c2_d = nc.dram_tensor("c2", [P, S], dt.bfloat16, kind="ExternalInput").ap()
    s2_d = nc.dram_tensor("s2", [P, S], dt.bfloat16, kind="ExternalInput").ap()
    perm_d = nc.dram_tensor("perm", [P, P], dt.bfloat16, kind="ExternalInput").ap()
    tri_d = nc.dram_tensor("tri", [P, P], dt.bfloat16, kind="ExternalInput").ap()
    out_d = nc.dram_tensor("out", [S, FPC], dt.float32, kind="ExternalOutput").ap()

    ts = bass.ts

    with tile.TileContext(nc) as tc:
        with (
            tc.tile_pool(name="const", bufs=1) as cpool,
            tc.tile_pool(name="work", bufs=4) as wpool,
            tc.tile_pool(name="small", bufs=6) as smpool,
            tc.tile_pool(name="ppsum", bufs=2,
                         space=bass.MemorySpace.PSUM) as ppsum,
            tc.tile_pool(name="spsum", bufs=3,
                         space=bass.MemorySpace.PSUM) as spsum,
            tc.tile_pool(name="opsum", bufs=1,
                         space=bass.MemorySpace.PSUM) as opsum,
            tc.tile_pool(name="ppool", bufs=8) as ppool,
        ):
            # --- persistent SBUF tensors ---
            wq_sb = cpool.tile([P, KT, FPC], dt.bfloat16, tag="wq")
            wk_sb = cpool.tile([P, KT, FPC], dt.bfloat16, tag="wk")
            wv_sb = cpool.tile([P, KT, FPC], dt.bfloat16, tag="wv")
            c2_sb = cpool.tile([P, S], dt.bfloat16, tag="c2")
            s2_sb = cpool.tile([P, S], dt.bfloat16, tag="s2")
            perm_sb = cpool.tile([P, P], dt.bfloat16, tag="perm")
            tri_sb = cpool.tile([P, P], dt.bfloat16, tag="tri")
            qT_sb = cpool.tile([P, HPC * S], dt.bfloat16, tag="qT")
            kT_sb = cpool.tile([P, HPC * S], dt.bfloat16, tag="kT")
            # v with ones column: index n = h*NQT + tt -> [128 tokens, 129]
            v_sb = cpool.tile([P, HPC * NQT * (DK + 1)], dt.bfloat16, tag="v")
            v_rr = v_sb.rearrange("p (n c) -> p n c", c=DK + 1)
            xts = [cpool.tile([P, KT, TBW], dt.bfloat16,
                              tag=f"xt{i}", name=f"xt{i}") for i in range(NTB)]

            # initial loads spread across engine queues for parallelism;
            # wq + xt0 first so the first projection chain starts early
            xT_r = xT_d.rearrange("(kt p) t -> p kt t", p=P)
            # wave 1: only what the first projection chain needs (wq head-0
            # columns + x t-block 0) so it starts ~12us in; everything else
            # queues behind it on the three DMA-capable engines
            wq_r = wqT_d.rearrange("(kt p) o -> p kt o", p=P)
            wk_r = wkT_d.rearrange("(kt p) o -> p kt o", p=P)
            wv_r = wvT_d.rearrange("(kt p) o -> p kt o", p=P)
            # x tiles split column-wise across the two HWDGE queues so each
            # tile completes in half the time; later-needed weights queue last
            HW = TBW // 2
            nc.sync.dma_start(wq_sb[:, :, 0:DK], wq_r[:, :, 0:DK])
            nc.scalar.dma_start(xts[0][:, :, 0:HW], xT_r[:, :, 0:HW])
            nc.sync.dma_start(xts[0][:, :, HW:TBW], xT_r[:, :, HW:TBW])
            nc.gpsimd.dma_start(c2_sb[:], c2_d[:])
            nc.gpsimd.dma_start(s2_sb[:], s2_d[:])
            nc.gpsimd.dma_start(perm_sb[:], perm_d[:])
            nc.gpsimd.dma_start(tri_sb[:], tri_d[:])
            nc.sync.dma_start(wk_sb[:, :, 0:DK], wk_r[:, :, 0:DK])
            for i in range(1, NTB):
                lo = i * TBW
                nc.scalar.dma_start(xts[i][:, :, 0:HW], xT_r[:, :, lo:lo + HW])
                nc.sync.dma_start(xts[i][:, :, HW:TBW],
                                  xT_r[:, :, lo + HW:lo + TBW])
            nc.scalar.dma_start(wk_sb[:, :, DK:FPC], wk_r[:, :, DK:FPC])
            nc.sync.dma_start(wv_sb[:], wv_r[:])
            nc.scalar.dma_start(wq_sb[:, :, DK:FPC], wq_r[:, :, DK:FPC])
            nc.gpsimd.memset(v_rr[:, :, DK:DK + 1], 1.0)

            # PE warm-up: dependency-free matmuls spin the tensor engine out
            # of its cold HAM state while the first input DMAs land
            warm_sb = cpool.tile([P, P], dt.bfloat16, tag="warm")
            nc.gpsimd.memset(warm_sb[:], 0.0)
            warm_ps = spsum.tile([P, P], dt.float32, tag="qtp",
                                 name="warm_ps", bufs=1)
            for _ in range(40):
                nc.tensor.matmul(warm_ps[:], warm_sb[:], warm_sb[:],
                                 start=True, stop=True)

            def v_proj():
                for tb in range(NTB):
                    for tsub in range(4):
                        tt = tb * 4 + tsub
                        vps = ppsum.tile([P, FPC], dt.float32,
                                         tag="qps", name="vps")
                        for kt in range(KT):
                            nc.tensor.matmul(
                                vps[:],
                                xts[tb][:, kt, ts(tsub, P)],
                                wv_sb[:, kt, :],
                                start=(kt == 0), stop=(kt == KT - 1),
                            )
                        for h in range(HPC):
                            nc.scalar.copy(v_rr[:, h * NQT + tt, 0:DK],
                                           vps[:, ts(h, DK)])

            def qk_proj(h):
                hq = h * S
                for tb in range(NTB):
                    for wsb, dest in ((wq_sb, qT_sb), (wk_sb, kT_sb)):
                        qps = ppsum.tile([P, TBW], dt.float32,
                                         tag="qps", name="qps")
                        for kt in range(KT):
                            nc.tensor.matmul(
                                qps[:],
                                wsb[:, kt, ts(h, DK)],
                                xts[tb][:, kt, :],
                                start=(kt == 0), stop=(kt == KT - 1),
                            )
                        qb_ = wpool.tile([P, TBW], dt.bfloat16,
                                         tag="qb", name="qb")
                        nc.scalar.copy(qb_[:], qps[:])
                        qtp = spsum.tile([P, TBW], dt.float32,
                                         tag="qtp", name="qtp", bufs=1)
                        nc.tensor.matmul(qtp[:], perm_sb[:], qb_[:],
                                         start=True, stop=True)
                        t1 = wpool.tile([P, TBW], dt.bfloat16,
                                        tag="t1", name="t1")
                        nc.vector.scalar_tensor_tensor(
                            t1[:], qb_[:], 1.0, c2_sb[:, ts(tb, TBW)],
                            op0=_mult, op1=_mult)
                        t2 = wpool.tile([P, TBW], dt.bfloat16,
                                        tag="t2", name="t2")
                        nc.vector.scalar_tensor_tensor(
                            t2[:], qtp[:], 1.0, s2_sb[:, ts(tb, TBW)],
                            op0=_mult, op1=_mult)
                        nc.vector.scalar_tensor_tensor(
                            dest[:, hq + tb * TBW:hq + (tb + 1) * TBW],
                            t1[:], 1.0, t2[:], op0=_mult, op1=_add)

            def attention(h):
                hq = h * S
                for qb in range(NTB):
                    # two accumulators share a PSUM bank: one start pending-
                    # zeroes the whole 2KB zero region, one stop (on the
                    # higher subtile, which always finishes later) releases it
                    o01 = opsum.tile([P, 2, DK + 1], dt.float32,
                                     tag="o01", name="o01")
                    o23 = opsum.tile([P, 2, DK + 1], dt.float32,
                                     tag="o23", name="o23")
                    oap = [o01[:, 0, :], o01[:, 1, :],
                           o23[:, 0, :], o23[:, 1, :]]
                    for kt in range(4 * qb + 4):
                        od = kt - 4 * qb  # >=0 on diagonal tiles
                        lo = max(od, 0) * P
                        sps = spsum.tile([P, TBW], dt.float32,
                                         tag="sps", name="sps")
                        nc.tensor.matmul(
                            sps[:, lo:TBW],
                            kT_sb[:, hq + kt * P:hq + (kt + 1) * P],
                            qT_sb[:, hq + qb * TBW + lo:hq + (qb + 1) * TBW],
                            start=True, stop=True)
                        pT = ppool.tile([P, TBW], dt.bfloat16,
                                        tag="pT", name="pT")
                        nc.scalar.activation(
                            pT[:, lo:TBW], sps[:, lo:TBW],
                            mybir.ActivationFunctionType.Exp)
                        if od >= 0:
                            nc.vector.scalar_tensor_tensor(
                                pT[:, od * P:(od + 1) * P],
                                pT[:, od * P:(od + 1) * P],
                                1.0, tri_sb[:], op0=_mult, op1=_mult)
                        for osub in range(max(od, 0), 4):
                            qt = 4 * qb + osub
                            nc.tensor.matmul(
                                oap[osub],
                                pT[:, osub * P:(osub + 1) * P],
                                v_rr[:, h * NQT + kt, :],
                                start=(kt == 0 and osub % 2 == 0),
                                stop=(kt == qt and osub % 2 == 1))
                    for osub in range(4):
                        qt = 4 * qb + osub
                        rec = smpool.tile([P, 1], dt.float32,
                                          tag="rec", name="rec")
                        nc.vector.reciprocal(rec[:], oap[osub][:, DK:DK + 1])
                        osb = smpool.tile([P, DK], dt.float32,
                                          tag="osb", name="osb")
                        nc.vector.tensor_scalar_mul(
                            osb[:], oap[osub][:, 0:DK], rec[:])
                        nc.sync.dma_start(out_d[ts(qt, P), ts(h, DK)], osb[:])

            qk_proj(0)
            v_proj()
            attention(0)
            for h in range(1, HPC):
                qk_proj(h)
                attention(h)

    nc.compile()
    return nc


def _host_tables():
    pos = np.arange(S, dtype=np.float64)
    i = np.arange(DK // 2, dtype=np.float64)
    inv_freq = THETA ** (-2.0 * i / DK)
    ang = pos[None, :] * inv_freq[:, None]          # [64, S]
    c2 = np.repeat(np.cos(ang), 2, axis=0).astype(bf16)   # [128, S]
    s2 = np.repeat(np.sin(ang), 2, axis=0).astype(bf16)
    perm = np.zeros((P, P), np.float32)
    idx = np.arange(DK // 2)
    perm[2 * idx + 1, 2 * idx] = -1.0
    perm[2 * idx, 2 * idx + 1] = 1.0
    tri = (np.arange(P)[:, None] <= np.arange(P)[None, :]).astype(np.float32)
    return c2, s2, perm.astype(bf16), tri.astype(bf16)


def kernel(x, wq, wk, wv):
    x = np.asarray(x, dtype=np.float32)
    wq = np.asarray(wq, dtype=np.float32)
    wk = np.asarray(wk, dtype=np.float32)
    wv = np.asarray(wv, dtype=np.float32)

    if "nc" not in _PROGRAM_CACHE:
        _PROGRAM_CACHE["nc"] = _build_program()
    nc = _PROGRAM_CACHE["nc"]

    c2, s2, perm, tri = _host_tables()
    scale = np.float32(1.0 / np.sqrt(DK))

    in_maps = []
    for c in range(N_CORES):
        b, hg = divmod(c, HPC)
        rows = slice(hg * FPC, (hg + 1) * FPC)
        in_maps.append({
            "xT": np.ascontiguousarray(x[b].T).astype(bf16),
            "wqT": np.ascontiguousarray((wq[rows] * scale).T).astype(bf16),
            "wkT": np.ascontiguousarray(wk[rows].T).astype(bf16),
            "wvT": np.ascontiguousarray(wv[rows].T).astype(bf16),
            "c2": c2, "s2": s2, "perm": perm, "tri": tri,
        })

    last_err = None
    for attempt in range(3):
        try:
            res = run_bass_kernel_spmd(nc, in_maps, list(range(N_CORES)),
                                       **_PROGRAM_CACHE.get("run_kwargs", {}))
            break
        except Exception as e:  # transient NRT device errors recover on retry
            last_err = e
            time.sleep(2.0)
    else:
        raise last_err
    _PROGRAM_CACHE["last_results"] = res

    out = np.empty((B, S, D), np.float32)
    for c in range(N_CORES):
        b, hg = divmod(c, HPC)
        out[b, :, hg * FPC:(hg + 1) * FPC] = res.results[c]["out"]
    return out
